# revision 1
# baseline (speedup 1.0000x reference)
"""GNN message-passing layer on 8 TRN2 NeuronCores.

Math: y[e] = relu(concat(x[i[e]], x[i[e]]) @ W1 + b1) @ W2 + b2
         = relu(x[i[e]] @ (W1[:C]+W1[C:]) + b1) @ W2 + b2.
The MLP depends only on the source node, so compute z = MLP(x) once per
node (50k rows), then y = z[nbr_idx] is a pure gather (800k rows).

Sharding: edges are split evenly across the 8 cores; each core computes
the full z table locally (x + weights replicated; phase A is tiny) and
then gathers + writes its own edge shard. No collectives.

Phase B uses the GPSIMD dma_gather custom instruction. Its indices are
signed int16, so the bf16 z table is gathered at pair-row granularity
(row = 2 nodes = 512B, pair id < 25088 fits int16); a DVE predicated
copy then selects the right half per edge (mask = idx & 1) and upcasts
to f32. Edge->position packing is chosen so the per-tile y write is one
contiguous 8KB descriptor per partition.
"""

from contextlib import ExitStack

import ml_dtypes
import numpy as np

import concourse.bacc as bacc
import concourse.mybir as mybir
import concourse.tile as tile
from concourse import library_config
from concourse.bass_utils import run_bass_kernel_spmd
from concourse.masks import make_identity

N_CORES = 8
C = 128  # channels (C_IN == C_OUT)
N_NODES = 50000
E_TOTAL = 800000

ACH = 512  # phase-A compute chunk (max moving dim per matmul)
SCH = 2048  # phase-A DMA super-chunk (one x load + one z write)
NPAD = ((N_NODES + SCH - 1) // SCH) * SCH  # 51200
NCH = NPAD // ACH  # 100

EPC = E_TOTAL // N_CORES  # 100000 edges per core
NI = 2048  # edges per dma_gather tile
TBB = (EPC + NI - 1) // NI  # 49 gather tiles
EPC_PAD = TBB * NI  # 100352
KCH = NI // 128  # 16 gathered chunks per partition

F32 = mybir.dt.float32
BF16 = mybir.dt.bfloat16

# matmul input dtype for phase A
MM_DT = mybir.dt.bfloat16


import os
PHASES = os.environ.get("KPHASES", "AB")


def _build_nc():
    nc = bacc.Bacc("TRN2", target_bir_lowering=False, debug=False,
                   num_devices=N_CORES, dynamic_dma_scratch_size=131072)

    xT = nc.dram_tensor("xT", [C, NPAD], BF16, kind="ExternalInput")
    idx16 = nc.dram_tensor("idx16", [128, EPC_PAD // 16], mybir.dt.int16,
                           kind="ExternalInput")
    parity = nc.dram_tensor("parity", [128, EPC_PAD // 128], mybir.dt.uint8,
                            kind="ExternalInput")
    w1 = nc.dram_tensor("w1", [C, C], BF16, kind="ExternalInput")
    w2 = nc.dram_tensor("w2", [C, C], BF16, kind="ExternalInput")
    b1 = nc.dram_tensor("b1", [C, 1], F32, kind="ExternalInput")
    b2 = nc.dram_tensor("b2", [C, 1], F32, kind="ExternalInput")
    y = nc.dram_tensor("y", [EPC_PAD, C], F32, kind="ExternalOutput")
    zkind = "ExternalOutput" if PHASES == "A" else \
        ("ExternalInput" if PHASES == "B" else "Internal")
    z = nc.dram_tensor("z_table", [NPAD, C], BF16, kind=zkind)

    with tile.TileContext(nc) as tc, ExitStack() as ctx:
        const = ctx.enter_context(tc.tile_pool(name="const", bufs=1))
        xpool = ctx.enter_context(tc.tile_pool(name="xin", bufs=2))
        hpool = ctx.enter_context(tc.tile_pool(name="hbuf", bufs=3))
        zb_pool = ctx.enter_context(tc.tile_pool(name="zb", bufs=3))
        gpool = ctx.enter_context(tc.tile_pool(name="gbuf", bufs=4))
        spool = ctx.enter_context(tc.tile_pool(name="sel", bufs=3))
        psA = ctx.enter_context(tc.tile_pool(name="psA", bufs=2, space="PSUM"))
        psT = ctx.enter_context(tc.tile_pool(name="psT", bufs=2, space="PSUM"))

        w1t = const.tile([C, C], MM_DT)
        w2t = const.tile([C, C], MM_DT)
        b1t = const.tile([C, 1], F32)
        b2t = const.tile([C, 1], F32)
        ident = const.tile([128, 128], BF16)
        idxt = const.tile([128, EPC_PAD // 16], mybir.dt.int16)
        maskt = const.tile([128, EPC_PAD // 128], mybir.dt.uint8)
        nc.sync.dma_start(out=w1t[:], in_=w1[:])
        nc.sync.dma_start(out=w2t[:], in_=w2[:])
        nc.sync.dma_start(out=b1t[:], in_=b1[:])
        nc.sync.dma_start(out=b2t[:], in_=b2[:])
        nc.sync.dma_start(out=idxt[:], in_=idx16[:])
        nc.sync.dma_start(out=maskt[:], in_=parity[:])
        make_identity(nc, ident[:])

        # ---- Phase A (skipped when PHASES=="B"):
        # z[n] = relu(x[n] @ W1eff + b1) @ W2 + b2 per 512-node chunk in
        # transposed orientation, PE-transposed back in 4-interleaved column
        # groups. DMA granularity is a 2048-node super-chunk: one x load and
        # one z write each.
        SUB = SCH // ACH
        for ts in range(NPAD // SCH if "A" in PHASES else 0):
            xt = xpool.tile([C, SCH], MM_DT)
            nc.sync.dma_start(out=xt[:], in_=xT[:, ts * SCH:(ts + 1) * SCH])
            # zbuf[q, b, j, c] = z[ts*SCH + 512b + 4q + j, c]
            zbuf = zb_pool.tile([128, SUB, ACH // 128, C], BF16, tag="zbuf")
            for b in range(SUB):
                h_ps = psA.tile([C, ACH], F32, tag="h_ps")
                nc.tensor.matmul(h_ps[:], w1t[:], xt[:, b * ACH:(b + 1) * ACH],
                                 start=True, stop=True)
                h_sb = hpool.tile([C, ACH], MM_DT, tag="h_sb")
                nc.scalar.activation(h_sb[:], h_ps[:],
                                     mybir.ActivationFunctionType.Relu,
                                     bias=b1t[:, 0:1])

                z_ps = psA.tile([C, ACH], F32, tag="z_ps")
                nc.tensor.matmul(z_ps[:], w2t[:], h_sb[:], start=True,
                                 stop=True)
                zt_sb = hpool.tile([C, ACH], BF16, tag="zt_sb")
                nc.vector.tensor_tensor(out=zt_sb[:], in0=z_ps[:],
                                        in1=b2t[:, 0:1].to_broadcast([C, ACH]),
                                        op=mybir.AluOpType.add)

                # transpose col group j (cols j, j+4, ...) -> rows +4q+j
                tr_ps = psT.tile([128, ACH // 128, 128], BF16, tag="tr")
                for j in range(ACH // 128):
                    nc.tensor.transpose(tr_ps[:, j, :], zt_sb[:, j:ACH:4],
                                        ident[:])
                if b % 2 == 0:
                    nc.vector.tensor_copy(zbuf[:, b, :, :], tr_ps[:])
                else:
                    nc.scalar.copy(zbuf[:, b, :, :], tr_ps[:])
            n0 = ts * SCH
            nc.scalar.dma_start(
                out=z[n0:n0 + SCH, :].rearrange("(b q j) c -> q b j c",
                                                b=SUB, j=4),
                in_=zbuf[:])

        tc.strict_bb_all_engine_barrier()

        # ---- Phase B: dma_gather pair-rows + DVE half-select, write y shard.
        nc.gpsimd.load_library(library_config.mlp)
        zview = z[:].rearrange("(a two) c -> a (two c)", two=2)  # [NPAD/2,2C]
        for t in range(TBB if "B" in PHASES else 0):
            g = gpool.tile([128, KCH, 2 * C], BF16, tag="g")
            nc.gpsimd.dma_gather(
                out_ap=g[:], in_ap=zview,
                idxs_ap=idxt[:, t * (NI // 16):(t + 1) * (NI // 16)],
                num_idxs=NI, num_idxs_reg=NI, elem_size=2 * C,
                single_packet=False)
            even = g[:, :, 0:C]
            odd = g[:, :, C:2 * C]
            m = maskt[:, t * KCH:(t + 1) * KCH].to_broadcast([128, KCH, C])
            nc.vector.copy_predicated(out=even, mask=m, data=odd)
            sel = spool.tile([128, KCH, C], F32, tag="sel")
            nc.scalar.copy(sel[:], even)
            # position (p, tl) holds edge row p*KCH + tl of this tile;
            # alternate the two HWDGE rings for the big y writes
            weng = nc.sync if t % 2 == 0 else nc.scalar
            weng.dma_start(
                out=y[t * NI:(t + 1) * NI, :].rearrange(
                    "(p tl) c -> p tl c", tl=KCH),
                in_=sel[:])

    nc.compile()
    return nc


_NC_CACHE = None


def _get_nc():
    global _NC_CACHE
    if _NC_CACHE is None:
        _NC_CACHE = _build_nc()
    return _NC_CACHE


def _pack_indices(idx_pad):
    """idx_pad: int32 [EPC_PAD] -> (idx16 [128, EPC_PAD//16] int16,
    parity [128, EPC_PAD//128] bf16) in the position layout where edge row
    r (within a tile) sits at gather position i = (r%16)*128 + r//16."""
    pair = (idx_pad >> 1).astype(np.int16)
    par = (idx_pad & 1).astype(np.uint8)

    r = np.arange(NI)
    pos = (r % 16) * 128 + r // 16  # position of row r

    pair_t = pair.reshape(TBB, NI)
    pair_by_pos = np.empty((TBB, NI), dtype=np.int16)
    pair_by_pos[:, pos] = pair_t
    # wrap: position i at [i%16, i//16] per tile, tiles side by side
    idx16 = (pair_by_pos.reshape(TBB, NI // 16, 16)
             .transpose(2, 0, 1).reshape(16, TBB * (NI // 16)))
    idx16 = np.tile(np.ascontiguousarray(idx16), (8, 1))

    # mask[p, t*KCH + tl] = parity of edge row p*KCH + tl of tile t
    mask = (par.reshape(TBB, 128, KCH).transpose(1, 0, 2)
            .reshape(128, TBB * KCH))
    return idx16, np.ascontiguousarray(mask).astype(np.uint8)


def kernel(x, nbr_idx, W1, b1, W2, b2, _trace=False, _trace_kwargs=None):
    x = np.asarray(x, dtype=np.float32)
    nbr_idx_np = np.asarray(nbr_idx)
    W1 = np.asarray(W1, dtype=np.float32)
    W2 = np.asarray(W2, dtype=np.float32)
    b1 = np.asarray(b1, dtype=np.float32)
    b2 = np.asarray(b2, dtype=np.float32)

    w1eff = np.ascontiguousarray(W1[:C] + W1[C:]).astype(ml_dtypes.bfloat16)
    w2_bf = W2.astype(ml_dtypes.bfloat16)
    xT = np.zeros((C, NPAD), dtype=ml_dtypes.bfloat16)
    xT[:, :N_NODES] = x.T.astype(ml_dtypes.bfloat16)

    in_maps = []
    for i in range(N_CORES):
        idx_pad = np.zeros(EPC_PAD, dtype=np.int32)
        idx_pad[:EPC] = nbr_idx_np[i * EPC:(i + 1) * EPC].astype(np.int32)
        idx16, mask = _pack_indices(idx_pad)
        in_maps.append({
            "xT": xT,
            "idx16": idx16,
            "parity": mask,
            "w1": w1eff,
            "w2": w2_bf,
            "b1": b1.reshape(C, 1),
            "b2": b2.reshape(C, 1),
        })

    nc = _get_nc()
    res = run_bass_kernel_spmd(nc, in_maps, list(range(N_CORES)),
                               trace=_trace, **(_trace_kwargs or {}))

    out = np.empty((E_TOTAL, C), dtype=np.float32)
    for i in range(N_CORES):
        out[i * EPC:(i + 1) * EPC] = res.results[i]["y"][:EPC]
    if _trace:
        return out, res
    return out



# revision 14
# speedup vs baseline: 1.3632x; 1.3632x over previous
"""GNN message-passing layer on 8 TRN2 NeuronCores.

Math: y[e] = relu(concat(x[i[e]], x[i[e]]) @ W1 + b1) @ W2 + b2
         = relu(x[i[e]] @ (W1[:C]+W1[C:]) + b1) @ W2 + b2.
The MLP depends only on the source node, so compute z = MLP(x) once per
node (50k rows), then y = z[nbr_idx] is a pure gather (800k rows).

Sharding: edges are split evenly across the 8 cores; each core computes
the full z table locally (x + weights replicated; phase A is tiny) and
then gathers + writes its own edge shard. No collectives.

Phase A: h^T = relu(W1eff^T x^T + b1) in column form (512-node moving
chunks, W1 stationary), then z in ROW form directly: one matmul per
128-node group with a stride-16 stationary slice of h^T, so group r of
a 2048-node super-chunk holds nodes {16p + r} and partition p's zbuf
row is 16 consecutive z rows -> 4KB-contiguous DRAM writes. No PE
transposes. b2 is added on the host (z gathers commute with +b2).

Phase B: GPSIMD dma_gather at pair-row granularity (signed int16
indices only reach 32767, so the bf16 z table is gathered as 512B rows
of 2 nodes; pair id < 25600 fits int16). A DVE predicated copy selects
the right half per edge (mask = idx & 1), a second DVE copy compacts to
a dense tile, and y is written bf16 with 4KB runs per partition. The
host upcasts to f32 (identical values to an on-chip upcast) and adds
b2.
"""

from contextlib import ExitStack

import ml_dtypes
import numpy as np

import concourse.bacc as bacc
import concourse.mybir as mybir
import concourse.tile as tile
from concourse import library_config
from concourse.bass_utils import run_bass_kernel_spmd

N_CORES = 8
C = 128  # channels (C_IN == C_OUT)
N_NODES = 50000
E_TOTAL = 800000

ACH = 512  # phase-A compute chunk (max moving dim per matmul)
SCH = 2048  # phase-A DMA super-chunk (one x load + one z write)
NPAD = ((N_NODES + SCH - 1) // SCH) * SCH  # 51200
GRP = SCH // 128  # 16 row-form matmul groups per super-chunk

import os
EPC = E_TOTAL // N_CORES  # 100000 edges per core
NI = int(os.environ.get("KNI", 2048))  # edges per gather tile
TBB = (EPC + NI - 1) // NI  # 49 gather tiles
EPC_PAD = TBB * NI  # 100352
KCH = NI // 128  # 16 gathered rows per partition per tile

F32 = mybir.dt.float32
BF16 = mybir.dt.bfloat16

# matmul input dtype for phase A
MM_DT = mybir.dt.bfloat16


PHASES = os.environ.get("KPHASES", "AB")


def _build_nc():
    nc = bacc.Bacc("TRN2", target_bir_lowering=False, debug=False,
                   num_devices=N_CORES, dynamic_dma_scratch_size=65536)

    xT = nc.dram_tensor("xT", [C, NPAD], BF16, kind="ExternalInput")
    idx16 = nc.dram_tensor("idx16", [128, EPC_PAD // 16], mybir.dt.int16,
                           kind="ExternalInput")
    parity = nc.dram_tensor("parity", [128, EPC_PAD // 128], mybir.dt.uint8,
                            kind="ExternalInput")
    w1 = nc.dram_tensor("w1", [C, C], BF16, kind="ExternalInput")
    w2 = nc.dram_tensor("w2", [C, C], BF16, kind="ExternalInput")
    b1 = nc.dram_tensor("b1", [C, 1], F32, kind="ExternalInput")
    y = nc.dram_tensor("y", [EPC_PAD, C], BF16, kind="ExternalOutput")
    zkind = "ExternalOutput" if PHASES == "A" else \
        ("ExternalInput" if PHASES == "B" else "Internal")
    z = nc.dram_tensor("z_table", [NPAD, C], BF16, kind=zkind)

    with tile.TileContext(nc) as tc, ExitStack() as ctx:
        B = lambda name, dflt: int(os.environ.get("KB_" + name, dflt))
        const = ctx.enter_context(tc.tile_pool(name="const", bufs=1))
        xpool = ctx.enter_context(tc.tile_pool(name="xin", bufs=B("x", 5)))
        hpool = ctx.enter_context(tc.tile_pool(name="hbuf", bufs=B("h", 3)))
        zb_pool = ctx.enter_context(tc.tile_pool(name="zb", bufs=B("zb", 5)))
        gpool = ctx.enter_context(tc.tile_pool(name="gbuf", bufs=B("g", 4)))
        spool = ctx.enter_context(tc.tile_pool(name="sel", bufs=B("s", 3)))
        psA = ctx.enter_context(
            tc.tile_pool(name="psA", bufs=B("pa", 3), space="PSUM"))
        psB = ctx.enter_context(
            tc.tile_pool(name="psB", bufs=B("pb", 3), space="PSUM"))

        w1t = const.tile([C, C], MM_DT)
        w2t = const.tile([C, C], MM_DT)
        b1t = const.tile([C, 1], F32)
        idxt = const.tile([128, EPC_PAD // 16], mybir.dt.int16)
        maskt = const.tile([128, EPC_PAD // 128], mybir.dt.uint8)
        nc.sync.dma_start(out=w1t[:], in_=w1[:])
        nc.sync.dma_start(out=b1t[:], in_=b1[:])
        nc.scalar.dma_start(out=w2t[:], in_=w2[:])
        nc.scalar.dma_start(out=idxt[:], in_=idx16[:])
        nc.scalar.dma_start(out=maskt[:], in_=parity[:])

        # ---- Phase A (skipped when PHASES=="B"). Emission is software-
        # pipelined one super-chunk deep, and the row-form matmuls of the
        # previous chunk are interleaved between the column-form matmuls
        # of the current chunk so the PE streams without engine gaps
        # (mm1 PSUM banks drain on ACT while the PE runs mm2s).
        NSC = NPAD // SCH if "A" in PHASES else 0
        NSC = int(os.environ.get("KNSC", NSC))

        # PE p-state warmup: stream dummy matmuls on the just-loaded weights
        # while the first x super-chunk is still in flight, so the tensor
        # engine enters the real pipeline already ramped to full clock.
        NWARM = int(os.environ.get("KWARM", 20))
        if NSC:
            warm = psA.tile([C, ACH], F32, tag="h_ps")
            for _ in range(NWARM):
                nc.tensor.matmul(warm[:, 0:C], w1t[:], w1t[:],
                                 start=True, stop=True)

        def chunk(ts, h_prev):
            xt = xpool.tile([C, SCH], MM_DT, tag="xt")
            nc.sync.dma_start(out=xt[:], in_=xT[:, ts * SCH:(ts + 1) * SCH])
            h_all = hpool.tile([C, SCH], MM_DT, tag="h")
            zbuf = None
            if h_prev is not None:
                # zbuf[p, r, c] = z[(ts-1)*SCH + 16p + r, c]
                zbuf = zb_pool.tile([128, GRP, C], BF16, tag="zbuf")
            for b in range(SCH // ACH):
                h_ps = psA.tile([C, ACH], F32, tag="h_ps")
                nc.tensor.matmul(h_ps[:], w1t[:],
                                 xt[:, b * ACH:(b + 1) * ACH],
                                 start=True, stop=True)
                if h_prev is not None:
                    z_ps = psB.tile([128, 4, C], F32, tag="z_ps")
                    for j in range(4):
                        r = b * 4 + j
                        nc.tensor.matmul(z_ps[:, j, :], h_prev[:, r::GRP],
                                         w2t[:], start=True, stop=True)
                    nc.vector.tensor_copy(zbuf[:, b * 4:(b + 1) * 4, :],
                                          z_ps[:])
                nc.scalar.activation(h_all[:, b * ACH:(b + 1) * ACH], h_ps[:],
                                     mybir.ActivationFunctionType.Relu,
                                     bias=b1t[:, 0:1])
            if h_prev is not None:
                n0 = (ts - 1) * SCH
                nc.sync.dma_start(
                    out=z[n0:n0 + SCH, :].rearrange("(p r) c -> p r c",
                                                    r=GRP),
                    in_=zbuf[:])
            return h_all

        def tailchunk(ts, h_prev):
            zbuf = zb_pool.tile([128, GRP, C], BF16, tag="zbuf")
            for q in range(GRP // 4):
                z_ps = psB.tile([128, 4, C], F32, tag="z_ps")
                for j in range(4):
                    r = q * 4 + j
                    nc.tensor.matmul(z_ps[:, j, :], h_prev[:, r::GRP],
                                     w2t[:], start=True, stop=True)
                nc.vector.tensor_copy(zbuf[:, q * 4:(q + 1) * 4, :], z_ps[:])
            n0 = ts * SCH
            nc.sync.dma_start(
                out=z[n0:n0 + SCH, :].rearrange("(p r) c -> p r c", r=GRP),
                in_=zbuf[:])

        prev = None
        for ts in range(NSC):
            prev = chunk(ts, prev)
        if prev is not None:
            tailchunk(NSC - 1, prev)

        tc.strict_bb_all_engine_barrier()

        # ---- Phase B: dma_gather pair rows, DVE half-select + compaction,
        # coalesced bf16 write. Edge e = t*NI + p*KCH + k sits at SBUF
        # [p, k, :] so each partition writes one contiguous 4KB run of y
        # rows per tile.
        if "B" in PHASES:
            nc.gpsimd.load_library(library_config.mlp)
        zview = z[:].rearrange("(a two) c -> a (two c)", two=2)  # [NPAD/2,2C]
        for t in range(TBB if "B" in PHASES else 0):
            g = gpool.tile([128, KCH, 2 * C], BF16, tag="g")
            nc.gpsimd.dma_gather(
                out_ap=g[:], in_ap=zview,
                idxs_ap=idxt[:, t * (NI // 16):(t + 1) * (NI // 16)],
                num_idxs=NI, num_idxs_reg=NI, elem_size=2 * C,
                single_packet=False)
            even = g[:, :, 0:C]
            odd = g[:, :, C:2 * C]
            m = maskt[:, t * KCH:(t + 1) * KCH].to_broadcast([128, KCH, C])
            nc.vector.copy_predicated(out=even, mask=m, data=odd)
            sel = spool.tile([128, KCH, C], BF16, tag="sel")
            nc.vector.tensor_copy(sel[:], even)
            # alternate the two HWDGE rings for the big y writes
            weng = nc.sync if t % 2 == 0 else nc.scalar
            weng.dma_start(
                out=y[t * NI:(t + 1) * NI, :].rearrange(
                    "(p k) c -> p k c", k=KCH),
                in_=sel[:])

    nc.compile()
    return nc


def _pack_indices(idx_pad):
    """idx_pad: int32 [EPC_PAD] -> (idx16 [128, EPC_PAD//16] int16,
    parity [128, EPC_PAD//128] uint8) in the position layout where edge row
    r (within a tile) sits at gather position i = (r%16)*128 + r//16, so
    edge t*NI + p*KCH + k lands at out [p, k, :]."""
    pair = (idx_pad >> 1).astype(np.int16)
    par = (idx_pad & 1).astype(np.uint8)

    r = np.arange(NI)
    pos = (r % 16) * 128 + r // 16  # position of row r

    pair_t = pair.reshape(TBB, NI)
    pair_by_pos = np.empty((TBB, NI), dtype=np.int16)
    pair_by_pos[:, pos] = pair_t
    # wrap: position i at [i%16, i//16] per tile, tiles side by side
    idx16 = (pair_by_pos.reshape(TBB, NI // 16, 16)
             .transpose(2, 0, 1).reshape(16, TBB * (NI // 16)))
    idx16 = np.tile(np.ascontiguousarray(idx16), (8, 1))

    # mask[p, t*KCH + k] = parity of edge row p*KCH + k of tile t
    mask = (par.reshape(TBB, 128, KCH).transpose(1, 0, 2)
            .reshape(128, TBB * KCH))
    return idx16, np.ascontiguousarray(mask).astype(np.uint8)


_NC_CACHE = None


def _get_nc():
    global _NC_CACHE
    if _NC_CACHE is None:
        _NC_CACHE = _build_nc()
    return _NC_CACHE


def kernel(x, nbr_idx, W1, b1, W2, b2, _trace=False, _trace_kwargs=None):
    x = np.asarray(x, dtype=np.float32)
    nbr_idx_np = np.asarray(nbr_idx)
    W1 = np.asarray(W1, dtype=np.float32)
    W2 = np.asarray(W2, dtype=np.float32)
    b1 = np.asarray(b1, dtype=np.float32)
    b2 = np.asarray(b2, dtype=np.float32)

    w1eff = np.ascontiguousarray(W1[:C] + W1[C:]).astype(ml_dtypes.bfloat16)
    w2_bf = W2.astype(ml_dtypes.bfloat16)
    xT = np.zeros((C, NPAD), dtype=ml_dtypes.bfloat16)
    xT[:, :N_NODES] = x.T.astype(ml_dtypes.bfloat16)

    in_maps = []
    for i in range(N_CORES):
        idx_pad = np.zeros(EPC_PAD, dtype=np.int32)
        idx_pad[:EPC] = nbr_idx_np[i * EPC:(i + 1) * EPC].astype(np.int32)
        idx16, mask = _pack_indices(idx_pad)
        in_maps.append({
            "xT": xT,
            "idx16": idx16,
            "parity": mask,
            "w1": w1eff,
            "w2": w2_bf,
            "b1": b1.reshape(C, 1),
        })

    nc = _get_nc()
    res = run_bass_kernel_spmd(nc, in_maps, list(range(N_CORES)),
                               trace=_trace, **(_trace_kwargs or {}))

    b2f = b2.astype(np.float32)
    out = np.empty((E_TOTAL, C), dtype=np.float32)
    for i in range(N_CORES):
        out[i * EPC:(i + 1) * EPC] = (
            res.results[i]["y"][:EPC].astype(np.float32) + b2f)
    if _trace:
        return out, res
    return out


# revision 25
# speedup vs baseline: 1.4126x; 1.0362x over previous
"""GNN message-passing layer on 8 TRN2 NeuronCores.

Math: y[e] = relu(concat(x[i[e]], x[i[e]]) @ W1 + b1) @ W2 + b2
         = relu(x[i[e]] @ (W1[:C]+W1[C:]) + b1) @ W2 + b2.
The MLP depends only on the source node, so compute z = MLP(x) once per
node (50k rows), then y = z[nbr_idx] is a pure gather (800k rows).

Sharding: edges are split evenly across the 8 cores; each core computes
the full z table locally (x + weights replicated; phase A is tiny) and
then gathers + writes its own edge shard. No collectives.

Phase A: h^T = relu(W1eff^T x^T + b1) in column form (512-node moving
chunks, W1 stationary), then z in ROW form directly: one matmul per
128-node group with a stride-16 stationary slice of h^T, so group r of
a 2048-node super-chunk holds nodes {16p + r} and partition p's zbuf
row is 16 consecutive z rows -> 4KB-contiguous DRAM writes. No PE
transposes. b2 is added on the host (z gathers commute with +b2).

Phase B: GPSIMD dma_gather at pair-row granularity (signed int16
indices only reach 32767, so the bf16 z table is gathered as 512B rows
of 2 nodes; pair id < 25600 fits int16). A DVE predicated copy selects
the right half per edge (mask = idx & 1), a second DVE copy compacts to
a dense tile, and y is written bf16 with 4KB runs per partition. The
host upcasts to f32 (identical values to an on-chip upcast) and adds
b2.
"""

from contextlib import ExitStack

import ml_dtypes
import numpy as np

import concourse.bacc as bacc
import concourse.mybir as mybir
import concourse.tile as tile
from concourse import library_config
from concourse.bass_utils import run_bass_kernel_spmd

N_CORES = 8
C = 128  # channels (C_IN == C_OUT)
N_NODES = 50000
E_TOTAL = 800000

ACH = 512  # phase-A compute chunk (max moving dim per matmul)
SCH = 2048  # phase-A DMA super-chunk (one x load + one z write)
NPAD = ((N_NODES + SCH - 1) // SCH) * SCH  # 51200
GRP = SCH // 128  # 16 row-form matmul groups per super-chunk

import os
EPC = E_TOTAL // N_CORES  # 100000 edges per core
NI = int(os.environ.get("KNI", 2048))  # edges per gather tile
TBB = (EPC + NI - 1) // NI  # 49 gather tiles
EPC_PAD = TBB * NI  # 100352
KCH = NI // 128  # 16 gathered rows per partition per tile

F32 = mybir.dt.float32
BF16 = mybir.dt.bfloat16

# matmul input dtype for phase A
MM_DT = mybir.dt.bfloat16


PHASES = os.environ.get("KPHASES", "AB")


def _build_nc():
    nc = bacc.Bacc("TRN2", target_bir_lowering=False, debug=False,
                   num_devices=N_CORES, dynamic_dma_scratch_size=65536)

    xT = nc.dram_tensor("xT", [C, NPAD], BF16, kind="ExternalInput")
    idx16 = nc.dram_tensor("idx16", [128, EPC_PAD // 16], mybir.dt.int16,
                           kind="ExternalInput")
    parity = nc.dram_tensor("parity", [128, EPC_PAD // 128], mybir.dt.uint8,
                            kind="ExternalInput")
    w1 = nc.dram_tensor("w1", [C, C], BF16, kind="ExternalInput")
    w2 = nc.dram_tensor("w2", [C, C], BF16, kind="ExternalInput")
    b1 = nc.dram_tensor("b1", [C, 1], F32, kind="ExternalInput")
    y = nc.dram_tensor("y", [EPC_PAD, C], BF16, kind="ExternalOutput")
    zkind = "ExternalOutput" if PHASES == "A" else \
        ("ExternalInput" if PHASES == "B" else "Internal")
    z = nc.dram_tensor("z_table", [NPAD, C], BF16, kind=zkind)

    with tile.TileContext(nc) as tc, ExitStack() as ctx:
        B = lambda name, dflt: int(os.environ.get("KB_" + name, dflt))
        const = ctx.enter_context(tc.tile_pool(name="const", bufs=1))
        xpool = ctx.enter_context(tc.tile_pool(name="xin", bufs=B("x", 5)))
        hpool = ctx.enter_context(tc.tile_pool(name="hbuf", bufs=B("h", 3)))
        zb_pool = ctx.enter_context(tc.tile_pool(name="zb", bufs=B("zb", 3)))
        gpool = ctx.enter_context(tc.tile_pool(name="gbuf", bufs=B("g", 4)))
        spool = ctx.enter_context(tc.tile_pool(name="sel", bufs=B("s", 3)))
        psA = ctx.enter_context(
            tc.tile_pool(name="psA", bufs=B("pa", 3), space="PSUM"))
        psB = ctx.enter_context(
            tc.tile_pool(name="psB", bufs=B("pb", 3), space="PSUM"))

        w1t = const.tile([C, C], MM_DT)
        w2t = const.tile([C, C], MM_DT)
        b1t = const.tile([C, 1], F32)
        idxt = const.tile([128, EPC_PAD // 16], mybir.dt.int16)
        maskt = const.tile([128, EPC_PAD // 128], mybir.dt.uint8)
        nc.scalar.dma_start(out=w1t[:], in_=w1[:])
        nc.scalar.dma_start(out=b1t[:], in_=b1[:])
        nc.scalar.dma_start(out=w2t[:], in_=w2[:])
        nc.scalar.dma_start(out=idxt[:], in_=idx16[:])
        nc.scalar.dma_start(out=maskt[:], in_=parity[:])

        # ---- Phase A (skipped when PHASES=="B"). Emission is software-
        # pipelined one super-chunk deep, and the row-form matmuls of the
        # previous chunk are interleaved between the column-form matmuls
        # of the current chunk so the PE streams without engine gaps
        # (mm1 PSUM banks drain on ACT while the PE runs mm2s). 4096-node
        # super-chunks keep each PE burst long enough to ramp the tensor
        # engine to its full p-state clock.
        SC1 = int(os.environ.get("KSC1", 4096))
        chunks = []
        if "A" in PHASES:
            # graded sizes: small first chunk to hide the initial x DMA
            # latency, small final chunks to shorten the pipeline drain
            KSZ = os.environ.get("KSZ", "512,1536|")
            head, tail = KSZ.split("|")
            head = [int(v) for v in head.split(",") if v]
            tail = [int(v) for v in tail.split(",") if v]
            mid = NPAD - sum(head) - sum(tail)
            assert mid % SC1 == 0, (head, tail, mid)
            sizes = head + [SC1] * (mid // SC1) + tail
            assert sum(sizes) == NPAD
            n0 = 0
            for sch in sizes:
                chunks.append((n0, sch))
                n0 += sch

        def zout(n0, grp, zbuf, qlo, qhi):
            # rows {grp*p + r}: contiguous per-partition runs
            nc.sync.dma_start(
                out=z[n0:n0 + grp * 128, :].rearrange(
                    "(p r) c -> p r c", r=grp)[:, qlo * 4:qhi * 4, :],
                in_=zbuf[:, qlo * 4:qhi * 4, :])

        def chunk(n0, sch, prev):
            xt = xpool.tile([C, SC1], MM_DT, tag="xt")
            nc.sync.dma_start(out=xt[:, 0:sch], in_=xT[:, n0:n0 + sch])
            h_all = hpool.tile([C, SC1], MM_DT, tag="h")
            zbuf = None
            if prev is not None:
                h_prev, n0p, schp = prev
                grpp = schp // 128
                zbuf = zb_pool.tile([128, SC1 // 128, C], BF16, tag="zbuf")
            nb = sch // ACH
            for b in range(nb):
                h_ps = psA.tile([C, ACH], F32, tag="h_ps")
                nc.tensor.matmul(h_ps[:], w1t[:],
                                 xt[:, b * ACH:(b + 1) * ACH],
                                 start=True, stop=True)
                if prev is not None:
                    qlo = (grpp * b) // (4 * nb)
                    qhi = (grpp * (b + 1)) // (4 * nb)
                    for q in range(qlo, qhi):
                        z_ps = psB.tile([128, 4, C], F32, tag="z_ps")
                        for j in range(4):
                            r = q * 4 + j
                            nc.tensor.matmul(z_ps[:, j, :],
                                             h_prev[:, r:schp:grpp], w2t[:],
                                             start=True, stop=True)
                        nc.vector.tensor_copy(
                            zbuf[:, q * 4:(q + 1) * 4, :], z_ps[:])
                nc.scalar.activation(h_all[:, b * ACH:(b + 1) * ACH], h_ps[:],
                                     mybir.ActivationFunctionType.Relu,
                                     bias=b1t[:, 0:1])
            if prev is not None:
                zout(n0p, grpp, zbuf, 0, grpp // 4)
            return (h_all, n0, sch)

        def tailchunk(prev):
            h_prev, n0, sch = prev
            grp = sch // 128
            zbuf = zb_pool.tile([128, SC1 // 128, C], BF16, tag="zbuf")
            for q in range(grp // 4):
                z_ps = psB.tile([128, 4, C], F32, tag="z_ps")
                for j in range(4):
                    r = q * 4 + j
                    nc.tensor.matmul(z_ps[:, j, :], h_prev[:, r:sch:grp],
                                     w2t[:], start=True, stop=True)
                nc.vector.tensor_copy(zbuf[:, q * 4:(q + 1) * 4, :], z_ps[:])
                # write as soon as computed to overlap the pipeline drain
                zout(n0, grp, zbuf, q, q + 1)

        prev = None
        for (n0, sch) in chunks:
            prev = chunk(n0, sch, prev)
        if prev is not None:
            tailchunk(prev)

        if not int(os.environ.get("KNOBAR", 0)):
            tc.strict_bb_all_engine_barrier()

        # ---- Phase B: dma_gather pair rows, DVE half-select + compaction,
        # coalesced bf16 write. Edge e = t*NI + p*KCH + k sits at SBUF
        # [p, k, :] so each partition writes one contiguous 4KB run of y
        # rows per tile.
        if "B" in PHASES:
            nc.gpsimd.load_library(library_config.mlp)
        zview = z[:].rearrange("(a two) c -> a (two c)", two=2)  # [NPAD/2,2C]
        for t in range(TBB if "B" in PHASES else 0):
            g = gpool.tile([128, KCH, 2 * C], BF16, tag="g")
            nc.gpsimd.dma_gather(
                out_ap=g[:], in_ap=zview,
                idxs_ap=idxt[:, t * (NI // 16):(t + 1) * (NI // 16)],
                num_idxs=NI, num_idxs_reg=NI, elem_size=2 * C,
                single_packet=False)
            even = g[:, :, 0:C]
            odd = g[:, :, C:2 * C]
            m = maskt[:, t * KCH:(t + 1) * KCH].to_broadcast([128, KCH, C])
            nc.vector.copy_predicated(out=even, mask=m, data=odd)
            sel = spool.tile([128, KCH, C], BF16, tag="sel")
            nc.vector.tensor_copy(sel[:], even)
            # alternate the two HWDGE rings for the big y writes
            weng = nc.sync if t % 2 == 0 else nc.scalar
            weng.dma_start(
                out=y[t * NI:(t + 1) * NI, :].rearrange(
                    "(p k) c -> p k c", k=KCH),
                in_=sel[:])

    nc.compile()
    return nc


def _pack_indices(idx_pad):
    """idx_pad: int32 [EPC_PAD] -> (idx16 [128, EPC_PAD//16] int16,
    parity [128, EPC_PAD//128] uint8) in the position layout where edge row
    r (within a tile) sits at gather position i = (r%16)*128 + r//16, so
    edge t*NI + p*KCH + k lands at out [p, k, :]."""
    pair = (idx_pad >> 1).astype(np.int16)
    par = (idx_pad & 1).astype(np.uint8)

    r = np.arange(NI)
    pos = (r % 16) * 128 + r // 16  # position of row r

    pair_t = pair.reshape(TBB, NI)
    pair_by_pos = np.empty((TBB, NI), dtype=np.int16)
    pair_by_pos[:, pos] = pair_t
    # wrap: position i at [i%16, i//16] per tile, tiles side by side
    idx16 = (pair_by_pos.reshape(TBB, NI // 16, 16)
             .transpose(2, 0, 1).reshape(16, TBB * (NI // 16)))
    idx16 = np.tile(np.ascontiguousarray(idx16), (8, 1))

    # mask[p, t*KCH + k] = parity of edge row p*KCH + k of tile t
    mask = (par.reshape(TBB, 128, KCH).transpose(1, 0, 2)
            .reshape(128, TBB * KCH))
    return idx16, np.ascontiguousarray(mask).astype(np.uint8)


_NC_CACHE = None


def _get_nc():
    global _NC_CACHE
    if _NC_CACHE is None:
        _NC_CACHE = _build_nc()
    return _NC_CACHE


def kernel(x, nbr_idx, W1, b1, W2, b2, _trace=False, _trace_kwargs=None):
    x = np.asarray(x, dtype=np.float32)
    nbr_idx_np = np.asarray(nbr_idx)
    W1 = np.asarray(W1, dtype=np.float32)
    W2 = np.asarray(W2, dtype=np.float32)
    b1 = np.asarray(b1, dtype=np.float32)
    b2 = np.asarray(b2, dtype=np.float32)

    w1eff = np.ascontiguousarray(W1[:C] + W1[C:]).astype(ml_dtypes.bfloat16)
    w2_bf = W2.astype(ml_dtypes.bfloat16)
    xT = np.zeros((C, NPAD), dtype=ml_dtypes.bfloat16)
    xT[:, :N_NODES] = x.T.astype(ml_dtypes.bfloat16)

    in_maps = []
    for i in range(N_CORES):
        idx_pad = np.zeros(EPC_PAD, dtype=np.int32)
        idx_pad[:EPC] = nbr_idx_np[i * EPC:(i + 1) * EPC].astype(np.int32)
        idx16, mask = _pack_indices(idx_pad)
        in_maps.append({
            "xT": xT,
            "idx16": idx16,
            "parity": mask,
            "w1": w1eff,
            "w2": w2_bf,
            "b1": b1.reshape(C, 1),
        })

    nc = _get_nc()
    res = run_bass_kernel_spmd(nc, in_maps, list(range(N_CORES)),
                               trace=_trace, **(_trace_kwargs or {}))

    b2f = b2.astype(np.float32)
    out = np.empty((E_TOTAL, C), dtype=np.float32)
    for i in range(N_CORES):
        out[i * EPC:(i + 1) * EPC] = (
            res.results[i]["y"][:EPC].astype(np.float32) + b2f)
    if _trace:
        return out, res
    return out


# revision 29
# speedup vs baseline: 1.4218x; 1.0065x over previous
"""GNN message-passing layer on 8 TRN2 NeuronCores.

Math: y[e] = relu(concat(x[i[e]], x[i[e]]) @ W1 + b1) @ W2 + b2
         = relu(x[i[e]] @ (W1[:C]+W1[C:]) + b1) @ W2 + b2.
The MLP depends only on the source node, so compute z = MLP(x) once per
node (50k rows), then y = z[nbr_idx] is a pure gather (800k rows).

Sharding: edges are split evenly across the 8 cores; each core computes
the full z table locally (x + weights replicated; phase A is tiny) and
then gathers + writes its own edge shard. No collectives.

Phase A: h^T = relu(W1eff^T x^T + b1) in column form (512-node moving
chunks, W1 stationary), then z in ROW form directly: one matmul per
128-node group with a stride-GRP stationary slice of h^T, so group r
of a super-chunk holds nodes {GRP*p + r} and partition p's zbuf row is
GRP consecutive z rows -> multi-KB contiguous DRAM writes. No PE
transposes; b2 is added on the host (z gathers commute with +b2).
Super-chunks are 4096 nodes (PE bursts long enough to ramp the tensor
engine to full clock) with smaller first/last chunks to shorten the
pipeline fill and drain; the row-form matmuls of each chunk are
interleaved between the column-form matmuls of the next.

Phase B: GPSIMD dma_gather at pair-row granularity (signed int16
indices only reach 32767, so the bf16 z table is gathered as 512B rows
of 2 nodes; pair id < 25600 fits int16). A DVE predicated copy selects
the right half per edge (mask = idx & 1), a second DVE copy compacts to
a dense tile, and y is written bf16 with 4KB runs per partition. The
host upcasts to f32 (identical values to an on-chip upcast) and adds
b2.
"""

from contextlib import ExitStack

import ml_dtypes
import numpy as np

import concourse.bacc as bacc
import concourse.mybir as mybir
import concourse.tile as tile
from concourse import library_config
from concourse.bass_utils import run_bass_kernel_spmd

N_CORES = 8
C = 128  # channels (C_IN == C_OUT)
N_NODES = 50000
E_TOTAL = 800000

ACH = 512  # phase-A compute chunk (max moving dim per matmul)
SCH = 2048  # phase-A DMA super-chunk (one x load + one z write)
NPAD = ((N_NODES + 511) // 512) * 512  # 50176
GRP = SCH // 128  # 16 row-form matmul groups per super-chunk

import os
EPC = E_TOTAL // N_CORES  # 100000 edges per core
NI = 2048  # edges per gather tile
TBB = (EPC + NI - 1) // NI  # 49 gather tiles
EPC_PAD = TBB * NI  # 100352
KCH = NI // 128  # 16 gathered rows per partition per tile
# phase-B tiles: uniform 2048-edge tiles with two 1024-edge tail tiles to
# shorten the end-of-kernel drain (gather -> select -> write chain)
BTILES = [(i * NI, NI) for i in range(TBB - 1)] + \
    [((TBB - 1) * NI, NI // 2), ((TBB - 1) * NI + NI // 2, NI // 2)]

F32 = mybir.dt.float32
BF16 = mybir.dt.bfloat16

# matmul input dtype for phase A
MM_DT = mybir.dt.bfloat16


PHASES = os.environ.get("KPHASES", "AB")


def _build_nc():
    nc = bacc.Bacc("TRN2", target_bir_lowering=False, debug=False,
                   num_devices=N_CORES, dynamic_dma_scratch_size=65536)

    xT = nc.dram_tensor("xT", [C, NPAD], BF16, kind="ExternalInput")
    idx16 = nc.dram_tensor("idx16", [128, EPC_PAD // 16], mybir.dt.int16,
                           kind="ExternalInput")
    parity = nc.dram_tensor("parity", [128, EPC_PAD // 128], mybir.dt.uint8,
                            kind="ExternalInput")
    w1 = nc.dram_tensor("w1", [C, C], BF16, kind="ExternalInput")
    w2 = nc.dram_tensor("w2", [C, C], BF16, kind="ExternalInput")
    b1 = nc.dram_tensor("b1", [C, 1], F32, kind="ExternalInput")
    y = nc.dram_tensor("y", [EPC_PAD, C], BF16, kind="ExternalOutput")
    zkind = "ExternalOutput" if PHASES == "A" else \
        ("ExternalInput" if PHASES == "B" else "Internal")
    z = nc.dram_tensor("z_table", [NPAD, C], BF16, kind=zkind)

    with tile.TileContext(nc) as tc, ExitStack() as ctx:
        const = ctx.enter_context(tc.tile_pool(name="const", bufs=1))
        xpool = ctx.enter_context(tc.tile_pool(name="xin", bufs=5))
        hpool = ctx.enter_context(tc.tile_pool(name="hbuf", bufs=3))
        zb_pool = ctx.enter_context(tc.tile_pool(name="zb", bufs=3))
        gpool = ctx.enter_context(tc.tile_pool(name="gbuf", bufs=5))
        spool = ctx.enter_context(tc.tile_pool(name="sel", bufs=4))
        psA = ctx.enter_context(tc.tile_pool(name="psA", bufs=3, space="PSUM"))
        psB = ctx.enter_context(tc.tile_pool(name="psB", bufs=3, space="PSUM"))

        w1t = const.tile([C, C], MM_DT)
        w2t = const.tile([C, C], MM_DT)
        b1t = const.tile([C, 1], F32)
        idxt = const.tile([128, EPC_PAD // 16], mybir.dt.int16)
        maskt = const.tile([128, EPC_PAD // 128], mybir.dt.uint8)
        nc.scalar.dma_start(out=w1t[:], in_=w1[:])
        nc.scalar.dma_start(out=b1t[:], in_=b1[:])
        nc.scalar.dma_start(out=w2t[:], in_=w2[:])
        nc.scalar.dma_start(out=idxt[:], in_=idx16[:])
        nc.scalar.dma_start(out=maskt[:], in_=parity[:])

        # ---- Phase A (skipped when PHASES=="B"). Emission is software-
        # pipelined one super-chunk deep, and the row-form matmuls of the
        # previous chunk are interleaved between the column-form matmuls
        # of the current chunk so the PE streams without engine gaps
        # (mm1 PSUM banks drain on ACT while the PE runs mm2s). 4096-node
        # super-chunks keep each PE burst long enough to ramp the tensor
        # engine to its full p-state clock.
        SC1 = 4096  # big chunks keep PE bursts long enough to reach full clock
        chunks = []
        if "A" in PHASES:
            # graded sizes: small first chunk to hide the initial x DMA
            # latency, small final chunks to shorten the pipeline drain
            sizes = [512, 1536] + [SC1] * 11 + [2048, 1024]
            assert sum(sizes) == NPAD
            n0 = 0
            for sch in sizes:
                chunks.append((n0, sch))
                n0 += sch

        def zout(n0, grp, zbuf, qlo, qhi):
            # rows {grp*p + r}: contiguous per-partition runs
            nc.sync.dma_start(
                out=z[n0:n0 + grp * 128, :].rearrange(
                    "(p r) c -> p r c", r=grp)[:, qlo * 4:qhi * 4, :],
                in_=zbuf[:, qlo * 4:qhi * 4, :])

        def chunk(n0, sch, prev):
            xt = xpool.tile([C, SC1], MM_DT, tag="xt")
            nc.sync.dma_start(out=xt[:, 0:sch], in_=xT[:, n0:n0 + sch])
            h_all = hpool.tile([C, SC1], MM_DT, tag="h")
            zbuf = None
            if prev is not None:
                h_prev, n0p, schp = prev
                grpp = schp // 128
                zbuf = zb_pool.tile([128, SC1 // 128, C], BF16, tag="zbuf")
            nb = sch // ACH
            for b in range(nb):
                h_ps = psA.tile([C, ACH], F32, tag="h_ps")
                nc.tensor.matmul(h_ps[:], w1t[:],
                                 xt[:, b * ACH:(b + 1) * ACH],
                                 start=True, stop=True)
                if prev is not None:
                    qlo = (grpp * b) // (4 * nb)
                    qhi = (grpp * (b + 1)) // (4 * nb)
                    for q in range(qlo, qhi):
                        z_ps = psB.tile([128, 4, C], F32, tag="z_ps")
                        for j in range(4):
                            r = q * 4 + j
                            nc.tensor.matmul(z_ps[:, j, :],
                                             h_prev[:, r:schp:grpp], w2t[:],
                                             start=True, stop=True)
                        nc.vector.tensor_copy(
                            zbuf[:, q * 4:(q + 1) * 4, :], z_ps[:])
                nc.scalar.activation(h_all[:, b * ACH:(b + 1) * ACH], h_ps[:],
                                     mybir.ActivationFunctionType.Relu,
                                     bias=b1t[:, 0:1])
            if prev is not None:
                zout(n0p, grpp, zbuf, 0, grpp // 4)
            return (h_all, n0, sch)

        def tailchunk(prev):
            h_prev, n0, sch = prev
            grp = sch // 128
            zbuf = zb_pool.tile([128, SC1 // 128, C], BF16, tag="zbuf")
            for q in range(grp // 4):
                z_ps = psB.tile([128, 4, C], F32, tag="z_ps")
                for j in range(4):
                    r = q * 4 + j
                    nc.tensor.matmul(z_ps[:, j, :], h_prev[:, r:sch:grp],
                                     w2t[:], start=True, stop=True)
                nc.vector.tensor_copy(zbuf[:, q * 4:(q + 1) * 4, :], z_ps[:])
                # write as soon as computed to overlap the pipeline drain
                zout(n0, grp, zbuf, q, q + 1)

        prev = None
        for (n0, sch) in chunks:
            prev = chunk(n0, sch, prev)
        if prev is not None:
            tailchunk(prev)

        tc.strict_bb_all_engine_barrier()

        # ---- Phase B: dma_gather pair rows, DVE half-select + compaction,
        # coalesced bf16 write. Edge e = t*NI + p*KCH + k sits at SBUF
        # [p, k, :] so each partition writes one contiguous 4KB run of y
        # rows per tile.
        if "B" in PHASES:
            nc.gpsimd.load_library(library_config.mlp)
        zview = z[:].rearrange("(a two) c -> a (two c)", two=2)  # [NPAD/2,2C]
        for t, (off, ni) in enumerate(BTILES if "B" in PHASES else []):
            ki = ni // 128
            g = gpool.tile([128, KCH, 2 * C], BF16, tag="g")
            nc.gpsimd.dma_gather(
                out_ap=g[:, 0:ki, :], in_ap=zview,
                idxs_ap=idxt[:, off // 16:(off + ni) // 16],
                num_idxs=ni, num_idxs_reg=ni, elem_size=2 * C,
                single_packet=False)
            even = g[:, 0:ki, 0:C]
            odd = g[:, 0:ki, C:2 * C]
            mo = off // 128
            m = maskt[:, mo:mo + ki].to_broadcast([128, ki, C])
            nc.vector.copy_predicated(out=even, mask=m, data=odd)
            sel = spool.tile([128, KCH, C], BF16, tag="sel")
            nc.vector.tensor_copy(sel[:, 0:ki, :], even)
            # alternate the two HWDGE rings for the big y writes
            weng = nc.sync if t % 2 == 0 else nc.scalar
            weng.dma_start(
                out=y[off:off + ni, :].rearrange("(p k) c -> p k c", k=ki),
                in_=sel[:, 0:ki, :])

    nc.compile()
    return nc


def _pack_indices(idx_pad):
    """idx_pad: int32 [EPC_PAD] -> (idx16 [128, EPC_PAD//16] int16,
    parity [128, EPC_PAD//128] uint8). Within a tile of ni edges, edge row
    r sits at gather position i = (r%ki)*128 + r//ki (ki = ni//128), so it
    lands at out [r//ki, r%ki, :]; gather positions wrap into 16 partitions
    ([i%16, i//16]), replicated across the 8 GPSIMD cores."""
    pair = (idx_pad >> 1).astype(np.int16)
    par = (idx_pad & 1).astype(np.uint8)

    idx_cols, mask_cols = [], []
    for off, ni in BTILES:
        ki = ni // 128
        r = np.arange(ni)
        pos = (r % ki) * 128 + r // ki
        pb = np.empty(ni, dtype=np.int16)
        pb[pos] = pair[off:off + ni]
        idx_cols.append(pb.reshape(ni // 16, 16).T)
        mask_cols.append(par[off:off + ni].reshape(128, ki))
    idx16 = np.tile(np.ascontiguousarray(np.concatenate(idx_cols, axis=1)),
                    (8, 1))
    mask = np.ascontiguousarray(np.concatenate(mask_cols, axis=1))
    return idx16, mask


_NC_CACHE = None


def _get_nc():
    global _NC_CACHE
    if _NC_CACHE is None:
        _NC_CACHE = _build_nc()
    return _NC_CACHE


def kernel(x, nbr_idx, W1, b1, W2, b2, _trace=False, _trace_kwargs=None):
    x = np.asarray(x, dtype=np.float32)
    nbr_idx_np = np.asarray(nbr_idx)
    W1 = np.asarray(W1, dtype=np.float32)
    W2 = np.asarray(W2, dtype=np.float32)
    b1 = np.asarray(b1, dtype=np.float32)
    b2 = np.asarray(b2, dtype=np.float32)

    w1eff = np.ascontiguousarray(W1[:C] + W1[C:]).astype(ml_dtypes.bfloat16)
    w2_bf = W2.astype(ml_dtypes.bfloat16)
    xT = np.zeros((C, NPAD), dtype=ml_dtypes.bfloat16)
    xT[:, :N_NODES] = x.T.astype(ml_dtypes.bfloat16)

    in_maps = []
    for i in range(N_CORES):
        idx_pad = np.zeros(EPC_PAD, dtype=np.int32)
        idx_pad[:EPC] = nbr_idx_np[i * EPC:(i + 1) * EPC].astype(np.int32)
        idx16, mask = _pack_indices(idx_pad)
        in_maps.append({
            "xT": xT,
            "idx16": idx16,
            "parity": mask,
            "w1": w1eff,
            "w2": w2_bf,
            "b1": b1.reshape(C, 1),
        })

    nc = _get_nc()
    res = run_bass_kernel_spmd(nc, in_maps, list(range(N_CORES)),
                               trace=_trace, **(_trace_kwargs or {}))

    b2f = b2.astype(np.float32)
    out = np.empty((E_TOTAL, C), dtype=np.float32)
    for i in range(N_CORES):
        out[i * EPC:(i + 1) * EPC] = (
            res.results[i]["y"][:EPC].astype(np.float32) + b2f)
    if _trace:
        return out, res
    return out


# revision 31
# speedup vs baseline: 1.4227x; 1.0006x over previous
"""GNN message-passing layer on 8 TRN2 NeuronCores.

Math: y[e] = relu(concat(x[i[e]], x[i[e]]) @ W1 + b1) @ W2 + b2
         = relu(x[i[e]] @ (W1[:C]+W1[C:]) + b1) @ W2 + b2.
The MLP depends only on the source node, so compute z = MLP(x) once per
node (50k rows), then y = z[nbr_idx] is a pure gather (800k rows).

Sharding: edges are split evenly across the 8 cores; each core computes
the full z table locally (x + weights replicated; phase A is tiny) and
then gathers + writes its own edge shard. No collectives.

Phase A: h^T = relu(W1eff^T x^T + b1) in column form (512-node moving
chunks, W1 stationary), then z in ROW form directly: one matmul per
128-node group with a stride-GRP stationary slice of h^T, so group r
of a super-chunk holds nodes {GRP*p + r} and partition p's zbuf row is
GRP consecutive z rows -> multi-KB contiguous DRAM writes. No PE
transposes; b2 is added on the host (z gathers commute with +b2).
Super-chunks are 4096 nodes (PE bursts long enough to ramp the tensor
engine to full clock) with smaller first/last chunks to shorten the
pipeline fill and drain; the row-form matmuls of each chunk are
interleaved between the column-form matmuls of the next.

Phase B: GPSIMD dma_gather at pair-row granularity (signed int16
indices only reach 32767, so the bf16 z table is gathered as 512B rows
of 2 nodes; pair id < 25088 fits int16). A DVE predicated copy selects
the right half per edge (mask = idx & 1), a second DVE copy compacts to
a dense tile, and y is written bf16 with 4KB runs per partition. The
host upcasts to f32 (identical values to an on-chip upcast) and adds
b2.
"""

from contextlib import ExitStack

import ml_dtypes
import numpy as np

import concourse.bacc as bacc
import concourse.mybir as mybir
import concourse.tile as tile
from concourse import library_config
from concourse.bass_utils import run_bass_kernel_spmd

N_CORES = 8
C = 128  # channels (C_IN == C_OUT)
N_NODES = 50000
E_TOTAL = 800000

ACH = 512  # phase-A compute chunk (max moving dim per matmul)
SCH = 2048  # phase-A DMA super-chunk (one x load + one z write)
NPAD = ((N_NODES + 511) // 512) * 512  # 50176
GRP = SCH // 128  # 16 row-form matmul groups per super-chunk

import os
EPC = E_TOTAL // N_CORES  # 100000 edges per core
NI = 2048  # edges per gather tile
TBB = (EPC + NI - 1) // NI  # 49 gather tiles
EPC_PAD = TBB * NI  # 100352
KCH = NI // 128  # 16 gathered rows per partition per tile
# phase-B tiles: uniform 2048-edge tiles with two 1024-edge tail tiles to
# shorten the end-of-kernel drain (gather -> select -> write chain)
BTILES = [(i * NI, NI) for i in range(TBB - 1)] + \
    [((TBB - 1) * NI, NI // 2), ((TBB - 1) * NI + NI // 2, NI // 2)]

F32 = mybir.dt.float32
BF16 = mybir.dt.bfloat16

# matmul input dtype for phase A
MM_DT = mybir.dt.bfloat16


PHASES = os.environ.get("KPHASES", "AB")


def _build_nc():
    nc = bacc.Bacc("TRN2", target_bir_lowering=False, debug=False,
                   num_devices=N_CORES, dynamic_dma_scratch_size=65536)

    xT = nc.dram_tensor("xT", [C, NPAD], BF16, kind="ExternalInput")
    idx16 = nc.dram_tensor("idx16", [128, EPC_PAD // 16], mybir.dt.int16,
                           kind="ExternalInput")
    parity = nc.dram_tensor("parity", [128, EPC_PAD // 128], mybir.dt.uint8,
                            kind="ExternalInput")
    w1 = nc.dram_tensor("w1", [C, C], BF16, kind="ExternalInput")
    w2 = nc.dram_tensor("w2", [C, C], BF16, kind="ExternalInput")
    b1 = nc.dram_tensor("b1", [C, 1], F32, kind="ExternalInput")
    y = nc.dram_tensor("y", [EPC_PAD, C], BF16, kind="ExternalOutput")
    zkind = "ExternalOutput" if PHASES == "A" else \
        ("ExternalInput" if PHASES == "B" else "Internal")
    z = nc.dram_tensor("z_table", [NPAD, C], BF16, kind=zkind)

    with tile.TileContext(nc) as tc, ExitStack() as ctx:
        const = ctx.enter_context(tc.tile_pool(name="const", bufs=1))
        xpool = ctx.enter_context(tc.tile_pool(name="xin", bufs=5))
        hpool = ctx.enter_context(tc.tile_pool(name="hbuf", bufs=3))
        zb_pool = ctx.enter_context(tc.tile_pool(name="zb", bufs=3))
        gpool = ctx.enter_context(tc.tile_pool(name="gbuf", bufs=5))
        spool = ctx.enter_context(tc.tile_pool(name="sel", bufs=4))
        psA = ctx.enter_context(tc.tile_pool(name="psA", bufs=4, space="PSUM"))
        psB = ctx.enter_context(tc.tile_pool(name="psB", bufs=4, space="PSUM"))

        w1t = const.tile([C, C], MM_DT)
        w2t = const.tile([C, C], MM_DT)
        b1t = const.tile([C, 1], F32)
        idxt = const.tile([128, EPC_PAD // 16], mybir.dt.int16)
        maskt = const.tile([128, EPC_PAD // 128], mybir.dt.uint8)
        nc.scalar.dma_start(out=w1t[:], in_=w1[:])
        nc.scalar.dma_start(out=b1t[:], in_=b1[:])
        nc.scalar.dma_start(out=w2t[:], in_=w2[:])
        nc.scalar.dma_start(out=idxt[:], in_=idx16[:])
        nc.scalar.dma_start(out=maskt[:], in_=parity[:])

        # ---- Phase A (skipped when PHASES=="B"). Emission is software-
        # pipelined one super-chunk deep, and the row-form matmuls of the
        # previous chunk are interleaved between the column-form matmuls
        # of the current chunk so the PE streams without engine gaps
        # (mm1 PSUM banks drain on ACT while the PE runs mm2s). 4096-node
        # super-chunks keep each PE burst long enough to ramp the tensor
        # engine to its full p-state clock.
        SC1 = 4096  # big chunks keep PE bursts long enough to reach full clock
        chunks = []
        if "A" in PHASES:
            # graded sizes: small first chunk to hide the initial x DMA
            # latency, small final chunks to shorten the pipeline drain
            sizes = [512, 1536] + [SC1] * 11 + [2048, 1024]
            assert sum(sizes) == NPAD
            n0 = 0
            for sch in sizes:
                chunks.append((n0, sch))
                n0 += sch

        def zout(n0, grp, zbuf, qlo, qhi):
            # rows {grp*p + r}: contiguous per-partition runs
            nc.sync.dma_start(
                out=z[n0:n0 + grp * 128, :].rearrange(
                    "(p r) c -> p r c", r=grp)[:, qlo * 4:qhi * 4, :],
                in_=zbuf[:, qlo * 4:qhi * 4, :])

        def chunk(n0, sch, prev):
            xt = xpool.tile([C, SC1], MM_DT, tag="xt")
            nc.sync.dma_start(out=xt[:, 0:sch], in_=xT[:, n0:n0 + sch])
            h_all = hpool.tile([C, SC1], MM_DT, tag="h")
            zbuf = None
            if prev is not None:
                h_prev, n0p, schp = prev
                grpp = schp // 128
                zbuf = zb_pool.tile([128, SC1 // 128, C], BF16, tag="zbuf")
            nb = sch // ACH
            for b in range(nb):
                h_ps = psA.tile([C, ACH], F32, tag="h_ps")
                nc.tensor.matmul(h_ps[:], w1t[:],
                                 xt[:, b * ACH:(b + 1) * ACH],
                                 start=True, stop=True)
                if prev is not None:
                    qlo = (grpp * b) // (4 * nb)
                    qhi = (grpp * (b + 1)) // (4 * nb)
                    for q in range(qlo, qhi):
                        z_ps = psB.tile([128, 4, C], F32, tag="z_ps")
                        for j in range(4):
                            r = q * 4 + j
                            nc.tensor.matmul(z_ps[:, j, :],
                                             h_prev[:, r:schp:grpp], w2t[:],
                                             start=True, stop=True)
                        nc.vector.tensor_copy(
                            zbuf[:, q * 4:(q + 1) * 4, :], z_ps[:])
                nc.scalar.activation(h_all[:, b * ACH:(b + 1) * ACH], h_ps[:],
                                     mybir.ActivationFunctionType.Relu,
                                     bias=b1t[:, 0:1])
            if prev is not None:
                zout(n0p, grpp, zbuf, 0, grpp // 4)
            return (h_all, n0, sch)

        def tailchunk(prev):
            h_prev, n0, sch = prev
            grp = sch // 128
            zbuf = zb_pool.tile([128, SC1 // 128, C], BF16, tag="zbuf")
            for q in range(grp // 4):
                z_ps = psB.tile([128, 4, C], F32, tag="z_ps")
                for j in range(4):
                    r = q * 4 + j
                    nc.tensor.matmul(z_ps[:, j, :], h_prev[:, r:sch:grp],
                                     w2t[:], start=True, stop=True)
                nc.vector.tensor_copy(zbuf[:, q * 4:(q + 1) * 4, :], z_ps[:])
                # write as soon as computed to overlap the pipeline drain
                zout(n0, grp, zbuf, q, q + 1)

        prev = None
        for (n0, sch) in chunks:
            prev = chunk(n0, sch, prev)
        if prev is not None:
            tailchunk(prev)

        tc.strict_bb_all_engine_barrier()

        # ---- Phase B: dma_gather pair rows, DVE half-select + compaction,
        # coalesced bf16 write. Edge e = t*NI + p*KCH + k sits at SBUF
        # [p, k, :] so each partition writes one contiguous 4KB run of y
        # rows per tile.
        if "B" in PHASES:
            nc.gpsimd.load_library(library_config.mlp)
        zview = z[:].rearrange("(a two) c -> a (two c)", two=2)  # [NPAD/2,2C]
        for t, (off, ni) in enumerate(BTILES if "B" in PHASES else []):
            ki = ni // 128
            g = gpool.tile([128, KCH, 2 * C], BF16, tag="g")
            nc.gpsimd.dma_gather(
                out_ap=g[:, 0:ki, :], in_ap=zview,
                idxs_ap=idxt[:, off // 16:(off + ni) // 16],
                num_idxs=ni, num_idxs_reg=ni, elem_size=2 * C,
                single_packet=False)
            even = g[:, 0:ki, 0:C]
            odd = g[:, 0:ki, C:2 * C]
            mo = off // 128
            m = maskt[:, mo:mo + ki].to_broadcast([128, ki, C])
            nc.vector.copy_predicated(out=even, mask=m, data=odd)
            sel = spool.tile([128, KCH, C], BF16, tag="sel")
            nc.vector.tensor_copy(sel[:, 0:ki, :], even)
            # alternate the two HWDGE rings for the big y writes
            weng = nc.sync if t % 2 == 0 else nc.scalar
            weng.dma_start(
                out=y[off:off + ni, :].rearrange("(p k) c -> p k c", k=ki),
                in_=sel[:, 0:ki, :])

    nc.compile()
    return nc


def _pack_indices(idx_pad):
    """idx_pad: int32 [EPC_PAD] -> (idx16 [128, EPC_PAD//16] int16,
    parity [128, EPC_PAD//128] uint8). Within a tile of ni edges, edge row
    r sits at gather position i = (r%ki)*128 + r//ki (ki = ni//128), so it
    lands at out [r//ki, r%ki, :]; gather positions wrap into 16 partitions
    ([i%16, i//16]), replicated across the 8 GPSIMD cores."""
    pair = (idx_pad >> 1).astype(np.int16)
    par = (idx_pad & 1).astype(np.uint8)

    idx_cols, mask_cols = [], []
    for off, ni in BTILES:
        ki = ni // 128
        r = np.arange(ni)
        pos = (r % ki) * 128 + r // ki
        pb = np.empty(ni, dtype=np.int16)
        pb[pos] = pair[off:off + ni]
        idx_cols.append(pb.reshape(ni // 16, 16).T)
        mask_cols.append(par[off:off + ni].reshape(128, ki))
    idx16 = np.tile(np.ascontiguousarray(np.concatenate(idx_cols, axis=1)),
                    (8, 1))
    mask = np.ascontiguousarray(np.concatenate(mask_cols, axis=1))
    return idx16, mask


_NC_CACHE = None


def _get_nc():
    global _NC_CACHE
    if _NC_CACHE is None:
        _NC_CACHE = _build_nc()
    return _NC_CACHE


def kernel(x, nbr_idx, W1, b1, W2, b2, _trace=False, _trace_kwargs=None):
    x = np.asarray(x, dtype=np.float32)
    nbr_idx_np = np.asarray(nbr_idx)
    W1 = np.asarray(W1, dtype=np.float32)
    W2 = np.asarray(W2, dtype=np.float32)
    b1 = np.asarray(b1, dtype=np.float32)
    b2 = np.asarray(b2, dtype=np.float32)

    w1eff = np.ascontiguousarray(W1[:C] + W1[C:]).astype(ml_dtypes.bfloat16)
    w2_bf = W2.astype(ml_dtypes.bfloat16)
    xT = np.zeros((C, NPAD), dtype=ml_dtypes.bfloat16)
    xT[:, :N_NODES] = x.T.astype(ml_dtypes.bfloat16)

    in_maps = []
    for i in range(N_CORES):
        idx_pad = np.zeros(EPC_PAD, dtype=np.int32)
        idx_pad[:EPC] = nbr_idx_np[i * EPC:(i + 1) * EPC].astype(np.int32)
        idx16, mask = _pack_indices(idx_pad)
        in_maps.append({
            "xT": xT,
            "idx16": idx16,
            "parity": mask,
            "w1": w1eff,
            "w2": w2_bf,
            "b1": b1.reshape(C, 1),
        })

    nc = _get_nc()
    res = run_bass_kernel_spmd(nc, in_maps, list(range(N_CORES)),
                               trace=_trace, **(_trace_kwargs or {}))

    b2f = b2.astype(np.float32)
    out = np.empty((E_TOTAL, C), dtype=np.float32)
    for i in range(N_CORES):
        out[i * EPC:(i + 1) * EPC] = (
            res.results[i]["y"][:EPC].astype(np.float32) + b2f)
    if _trace:
        return out, res
    return out


# revision 51
# speedup vs baseline: 1.4320x; 1.0065x over previous
"""GNN message-passing layer on 8 TRN2 NeuronCores.

Math: y[e] = relu(concat(x[i[e]], x[i[e]]) @ W1 + b1) @ W2 + b2
         = relu(x[i[e]] @ (W1[:C]+W1[C:]) + b1) @ W2 + b2.
The MLP depends only on the source node, so compute z = MLP(x) once per
node (50k rows), then y = z[nbr_idx] is a pure gather (800k rows).

Sharding: edges are split evenly across the 8 cores; each core computes
the full z table locally (x + weights replicated; phase A is tiny) and
then gathers + writes its own edge shard. No collectives.

Phase A: h^T = relu(W1eff^T x^T + b1) in column form (512-node moving
chunks, W1 stationary), then z in ROW form directly: one matmul per
128-node group with a stride-GRP stationary slice of h^T, so group r
of a super-chunk holds nodes {GRP*p + r} and partition p's zbuf row is
GRP consecutive z rows -> multi-KB contiguous DRAM writes. No PE
transposes; b2 is added on the host (z gathers commute with +b2).
Super-chunks are 4096 nodes (PE bursts long enough to ramp the tensor
engine to full clock) with smaller first/last chunks to shorten the
pipeline fill and drain; the row-form matmuls of each chunk are
interleaved between the column-form matmuls of the next.

Phase B: GPSIMD dma_gather at pair-row granularity (signed int16
indices only reach 32767, so the bf16 z table is gathered as 512B rows
of 2 nodes; pair id < 25088 fits int16). A DVE predicated copy selects
the right half per edge (mask = idx & 1), a second DVE copy compacts to
a dense tile, and y is written bf16 with 4KB runs per partition. The
host upcasts to f32 (identical values to an on-chip upcast) and adds
b2.
"""

from contextlib import ExitStack

import ml_dtypes
import numpy as np

import concourse.bacc as bacc
import concourse.mybir as mybir
import concourse.tile as tile
from concourse import library_config
from concourse.bass_utils import run_bass_kernel_spmd

N_CORES = 8
C = 128  # channels (C_IN == C_OUT)
N_NODES = 50000
E_TOTAL = 800000

ACH = 512  # phase-A compute chunk (max moving dim per matmul)
SCH = 2048  # phase-A DMA super-chunk (one x load + one z write)
NPAD = ((N_NODES + 511) // 512) * 512  # 50176
GRP = SCH // 128  # 16 row-form matmul groups per super-chunk

import os
EPC = E_TOTAL // N_CORES  # 100000 edges per core
NI = 2048  # edges per gather tile
TBB = (EPC + NI - 1) // NI  # 49 gather tiles
EPC_PAD = TBB * NI  # 100352
KCH = NI // 128  # 16 gathered rows per partition per tile
# phase-B tiles: uniform 2048-edge tiles with two 1024-edge tail tiles to
# shorten the end-of-kernel drain (gather -> select -> write chain)
def _btiles():
    # graded: small head tiles so the first gather's descriptor generation
    # (and thus the first post-barrier DMA) starts sooner; small tail tiles
    # to shorten the end-of-kernel gather->select->write drain
    sizes = [NI] * 48 + [1024, 1024]
    assert sum(sizes) == EPC_PAD
    out, off = [], 0
    for ni in sizes:
        out.append((off, ni))
        off += ni
    return out


BTILES = _btiles()

F32 = mybir.dt.float32
BF16 = mybir.dt.bfloat16

# matmul input dtype for phase A
MM_DT = mybir.dt.bfloat16


PHASES = os.environ.get("KPHASES", "AB")


def _build_nc():
    nc = bacc.Bacc("TRN2", target_bir_lowering=False, debug=False,
                   num_devices=N_CORES, dynamic_dma_scratch_size=65536)

    xT = nc.dram_tensor("xT", [C, NPAD], BF16, kind="ExternalInput")
    idx16 = nc.dram_tensor("idx16", [128, EPC_PAD // 16], mybir.dt.int16,
                           kind="ExternalInput")
    parity = nc.dram_tensor("parity", [128, EPC_PAD // 128], mybir.dt.uint8,
                            kind="ExternalInput")
    w1 = nc.dram_tensor("w1", [C, C], BF16, kind="ExternalInput")
    w2 = nc.dram_tensor("w2", [C, C], BF16, kind="ExternalInput")
    b1 = nc.dram_tensor("b1", [C, 1], F32, kind="ExternalInput")
    y = nc.dram_tensor("y", [EPC_PAD, C], BF16, kind="ExternalOutput")
    zkind = "ExternalOutput" if PHASES == "A" else \
        ("ExternalInput" if PHASES == "B" else "Internal")
    z = nc.dram_tensor("z_table", [NPAD, C], BF16, kind=zkind)

    with tile.TileContext(nc) as tc, ExitStack() as ctx:
        const = ctx.enter_context(tc.tile_pool(name="const", bufs=1))
        xpool = ctx.enter_context(tc.tile_pool(name="xin", bufs=4))
        hpool = ctx.enter_context(tc.tile_pool(name="hbuf", bufs=3))
        zb_pool = ctx.enter_context(tc.tile_pool(name="zb", bufs=4))
        gpool = ctx.enter_context(tc.tile_pool(name="gbuf", bufs=5))
        spool = ctx.enter_context(tc.tile_pool(name="sel", bufs=4))
        psA = ctx.enter_context(tc.tile_pool(name="psA", bufs=4, space="PSUM"))
        psB = ctx.enter_context(tc.tile_pool(name="psB", bufs=4, space="PSUM"))

        w1t = const.tile([C, C], MM_DT)
        w2t = const.tile([C, C], MM_DT)
        b1t = const.tile([C, 1], F32)
        idxt = const.tile([128, EPC_PAD // 16], mybir.dt.int16)
        maskt = const.tile([128, EPC_PAD // 128], mybir.dt.uint8)
        if "B" in PHASES:
            nc.gpsimd.load_library(library_config.mlp)
        nc.scalar.dma_start(out=w1t[:], in_=w1[:])
        nc.scalar.dma_start(out=b1t[:], in_=b1[:])
        nc.scalar.dma_start(out=w2t[:], in_=w2[:])
        nc.scalar.dma_start(out=idxt[:], in_=idx16[:])
        nc.scalar.dma_start(out=maskt[:], in_=parity[:])

        # ---- Phase A (skipped when PHASES=="B"). Emission is software-
        # pipelined one super-chunk deep, and the row-form matmuls of the
        # previous chunk are interleaved between the column-form matmuls
        # of the current chunk so the PE streams without engine gaps
        # (mm1 PSUM banks drain on ACT while the PE runs mm2s). 4096-node
        # super-chunks keep each PE burst long enough to ramp the tensor
        # engine to its full p-state clock.
        SC1 = 4096  # big chunks keep PE bursts long enough to reach full clock
        chunks = []
        if "A" in PHASES:
            # graded sizes: small first chunk to hide the initial x DMA
            # latency, small final chunks to shorten the pipeline drain
            sizes = [512, 1536] + [SC1] * 11 + [2048, 512, 512]
            assert sum(sizes) == NPAD
            n0 = 0
            for sch in sizes:
                chunks.append((n0, sch))
                n0 += sch

        def zout(n0, grp, zbuf, qlo, qhi):
            # rows {grp*p + r}: contiguous per-partition runs
            nc.sync.dma_start(
                out=z[n0:n0 + grp * 128, :].rearrange(
                    "(p r) c -> p r c", r=grp)[:, qlo * 4:qhi * 4, :],
                in_=zbuf[:, qlo * 4:qhi * 4, :])

        def chunk(n0, sch, prev, late=False):
            xt = xpool.tile([C, SC1], MM_DT, tag="xt")
            nc.sync.dma_start(out=xt[:, 0:sch], in_=xT[:, n0:n0 + sch])
            h_all = hpool.tile([C, SC1], MM_DT, tag="h")
            zbuf = None
            if prev is not None:
                h_prev, n0p, schp = prev
                grpp = schp // 128
                zbuf = zb_pool.tile([128, SC1 // 128, C], BF16, tag="zbuf")
            nb = sch // ACH
            for b in range(nb):
                h_ps = psA.tile([C, ACH], F32, tag="h_ps")
                nc.tensor.matmul(h_ps[:], w1t[:],
                                 xt[:, b * ACH:(b + 1) * ACH],
                                 start=True, stop=True)
                if prev is not None:
                    qlo = (grpp * b) // (4 * nb)
                    qhi = (grpp * (b + 1)) // (4 * nb)
                    for q in range(qlo, qhi):
                        z_ps = psB.tile([128, 4, C], F32, tag="z_ps")
                        for j in range(4):
                            r = q * 4 + j
                            nc.tensor.matmul(z_ps[:, j, :],
                                             h_prev[:, r:schp:grpp], w2t[:],
                                             start=True, stop=True)
                        if late and q % 2 == 1:
                            nc.scalar.copy(zbuf[:, q * 4:(q + 1) * 4, :],
                                           z_ps[:])
                        else:
                            nc.vector.tensor_copy(
                                zbuf[:, q * 4:(q + 1) * 4, :], z_ps[:])
                nc.scalar.activation(h_all[:, b * ACH:(b + 1) * ACH], h_ps[:],
                                     mybir.ActivationFunctionType.Relu,
                                     bias=b1t[:, 0:1])
            if prev is not None:
                zout(n0p, grpp, zbuf, 0, grpp // 4)
            return (h_all, n0, sch)

        def tailchunk(prev):
            h_prev, n0, sch = prev
            grp = sch // 128
            zbuf = zb_pool.tile([128, SC1 // 128, C], BF16, tag="zbuf")
            for q in range(grp // 4):
                z_ps = psB.tile([128, 4, C], F32, tag="z_ps")
                for j in range(4):
                    r = q * 4 + j
                    nc.tensor.matmul(z_ps[:, j, :], h_prev[:, r:sch:grp],
                                     w2t[:], start=True, stop=True)
                nc.scalar.copy(zbuf[:, q * 4:(q + 1) * 4, :], z_ps[:])
                # write as soon as computed to overlap the pipeline drain
                zout(n0, grp, zbuf, q, q + 1)

        prev = None
        for ci, (n0, sch) in enumerate(chunks):
            prev = chunk(n0, sch, prev, late=(ci >= len(chunks) - 3))
        if prev is not None:
            tailchunk(prev)

        zview = z[:].rearrange("(a two) c -> a (two c)", two=2)  # [NPAD/2,2C]

        # No explicit barrier between the phases: the gathers' reads of the
        # z table carry tracked RAW sync deps on all of phase A's z writes,
        # which is the exact ordering required (verified: without it phase B
        # still starts only after the final z write lands).

        # ---- Phase B: dma_gather pair rows, DVE half-select + compaction,
        # coalesced bf16 write. Edge e = t*NI + p*KCH + k sits at SBUF
        # [p, k, :] so each partition writes one contiguous 4KB run of y
        # rows per tile.
        for t, (off, ni) in enumerate(BTILES if "B" in PHASES else []):
            ki = ni // 128
            g = gpool.tile([128, KCH, 2 * C], BF16, tag="g")
            nc.gpsimd.dma_gather(
                out_ap=g[:, 0:ki, :], in_ap=zview,
                idxs_ap=idxt[:, off // 16:(off + ni) // 16],
                num_idxs=ni, num_idxs_reg=ni, elem_size=2 * C,
                single_packet=False)
            even = g[:, 0:ki, 0:C]
            odd = g[:, 0:ki, C:2 * C]
            mo = off // 128
            m = maskt[:, mo:mo + ki].to_broadcast([128, ki, C])
            nc.vector.copy_predicated(out=even, mask=m, data=odd)
            sel = spool.tile([128, KCH, C], BF16, tag="sel")
            nc.vector.tensor_copy(sel[:, 0:ki, :], even)
            # alternate the two HWDGE rings for the big y writes
            weng = nc.sync if t % 2 == 0 else nc.scalar
            weng.dma_start(
                out=y[off:off + ni, :].rearrange("(p k) c -> p k c", k=ki),
                in_=sel[:, 0:ki, :])

    nc.compile()
    return nc


def _pack_indices(idx_pad):
    """idx_pad: int32 [EPC_PAD] -> (idx16 [128, EPC_PAD//16] int16,
    parity [128, EPC_PAD//128] uint8). Within a tile of ni edges, edge row
    r sits at gather position i = (r%ki)*128 + r//ki (ki = ni//128), so it
    lands at out [r//ki, r%ki, :]; gather positions wrap into 16 partitions
    ([i%16, i//16]), replicated across the 8 GPSIMD cores."""
    pair = (idx_pad >> 1).astype(np.int16)
    par = (idx_pad & 1).astype(np.uint8)

    idx_cols, mask_cols = [], []
    for off, ni in BTILES:
        ki = ni // 128
        r = np.arange(ni)
        pos = (r % ki) * 128 + r // ki
        pb = np.empty(ni, dtype=np.int16)
        pb[pos] = pair[off:off + ni]
        idx_cols.append(pb.reshape(ni // 16, 16).T)
        mask_cols.append(par[off:off + ni].reshape(128, ki))
    idx16 = np.tile(np.ascontiguousarray(np.concatenate(idx_cols, axis=1)),
                    (8, 1))
    mask = np.ascontiguousarray(np.concatenate(mask_cols, axis=1))
    return idx16, mask


_NC_CACHE = None


def _get_nc():
    global _NC_CACHE
    if _NC_CACHE is None:
        _NC_CACHE = _build_nc()
    return _NC_CACHE


def kernel(x, nbr_idx, W1, b1, W2, b2, _trace=False, _trace_kwargs=None):
    x = np.asarray(x, dtype=np.float32)
    nbr_idx_np = np.asarray(nbr_idx)
    W1 = np.asarray(W1, dtype=np.float32)
    W2 = np.asarray(W2, dtype=np.float32)
    b1 = np.asarray(b1, dtype=np.float32)
    b2 = np.asarray(b2, dtype=np.float32)

    w1eff = np.ascontiguousarray(W1[:C] + W1[C:]).astype(ml_dtypes.bfloat16)
    w2_bf = W2.astype(ml_dtypes.bfloat16)
    xT = np.zeros((C, NPAD), dtype=ml_dtypes.bfloat16)
    xT[:, :N_NODES] = x.T.astype(ml_dtypes.bfloat16)

    in_maps = []
    for i in range(N_CORES):
        idx_pad = np.zeros(EPC_PAD, dtype=np.int32)
        idx_pad[:EPC] = nbr_idx_np[i * EPC:(i + 1) * EPC].astype(np.int32)
        idx16, mask = _pack_indices(idx_pad)
        in_maps.append({
            "xT": xT,
            "idx16": idx16,
            "parity": mask,
            "w1": w1eff,
            "w2": w2_bf,
            "b1": b1.reshape(C, 1),
        })

    nc = _get_nc()
    res = run_bass_kernel_spmd(nc, in_maps, list(range(N_CORES)),
                               trace=_trace, **(_trace_kwargs or {}))

    b2f = b2.astype(np.float32)
    out = np.empty((E_TOTAL, C), dtype=np.float32)
    for i in range(N_CORES):
        out[i * EPC:(i + 1) * EPC] = (
            res.results[i]["y"][:EPC].astype(np.float32) + b2f)
    if _trace:
        return out, res
    return out
